# revision 12
# baseline (speedup 1.0000x reference)
"""MoE feed-forward kernel for 8 Trainium2 NeuronCores.

Strategy (v2):
  - Router (tiny: x @ rW, top-2, softmax) runs on host in numpy.
  - Expert-parallel: core e owns routed expert e. Host gathers the tokens
    routed to expert e, padded only to R = max_e count_e (NOT to a
    multiple of 128): the routed phase streams exact token counts.
  - Routed GEMM2 runs in the TRANSPOSED orientation: stationary = W2
    tile [128 h, 128 d], moving = ht [128 h, tokens]. Output is
    [128 d-partitions, tokens] so cost scales with the token count, not
    with 128-aligned token blocks; the 1152-padding of v1 is gone.
    The gate multiply moves to the host scatter-add (output is raw y).
  - W1 and W2 are STREAMED through small 3-slot SBUF rings (stationary
    tiles are each used once per output pass), freeing ~48KB/partition.
  - Shared experts: sharded (expert s = core//4, hidden-quarter q =
    core%4); each core computes its quarter over all tokens; host sums
    the 8 partials (0.5 mean factor folded into sW2 upload).
  - All matmul operands bf16 (PSUM accumulates fp32).
  - Startup: sW1 is loaded in per-h 256KB chunks so the first shared
    matmul only waits for 1/8 of the weights; the warmup is 8 dummy
    matmuls (~1.7us, sized to the DMA prefix) plus one biased GELU so
    both scalar-engine activation tables load during the DMA wait.
  - Every input is pre-packed on the host into the exact SBUF tile
    layout (partition-major) so each DMA moves contiguous KBs per
    partition. Outputs are stored bf16.
"""

import sys
import types

import numpy as np
import ml_dtypes

sys.path.insert(0, "/opt/trn_rl_repo")

import concourse.bass as bass  # noqa: E402
import concourse.mybir as mybir  # noqa: E402
import concourse.tile as tile  # noqa: E402
from concourse import bacc  # noqa: E402
from concourse.bass_utils import run_bass_kernel_spmd  # noqa: E402

F32 = mybir.dt.float32
BF16 = mybir.dt.bfloat16
NPBF16 = ml_dtypes.bfloat16
GELU = mybir.ActivationFunctionType.Gelu

D = 1024      # d_model
H = 4096      # expert hidden
HQ = 1024     # shared-expert hidden slice per core (H / 4)
T = 4096      # tokens (2 * 2048)
E = 8         # routed experts
TOP_K = 2
NCORES = 8
NCB = T // 512  # token blocks in the shared phase


def _install_ntff_hook():
    """Shim for the missing antenv.axon_hooks so trace=True can profile."""
    try:
        import antenv
        if "antenv.axon_hooks" in sys.modules:
            return
        mod = types.ModuleType("antenv.axon_hooks")
        mod._hook = None
        mod.set_axon_ntff_profile_hook = lambda h: setattr(mod, "_hook", h)
        mod.get_axon_ntff_profile_hook = lambda: mod._hook
        sys.modules["antenv.axon_hooks"] = mod
        antenv.axon_hooks = mod
        sys.path.insert(0, "/root/.axon_site/trn_agent_boot")
        import trn_boot
        hook = trn_boot._ntff_profile_via_ctypes("/opt/axon/libaxon_pjrt.so")
        mod.set_axon_ntff_profile_hook(hook)
    except Exception:
        pass


def _split_groups(r):
    """Split r tokens into balanced moving-dim groups of <=512 (each >=64
    so LDWEIGHTS stays hidden under the stream)."""
    n = (r + 511) // 512
    base = r // n
    rem = r - base * n
    out = []
    start = 0
    for i in range(n):
        sz = base + (1 if i < rem else 0)
        out.append((start, sz))
        start += sz
    return out


def _pack(mat, inner):
    """(R, cols) -> (128, R//128, cols...) partition-major bf16 host packing:
    out[p, a, ...] = mat[a*128 + p, ...]."""
    r = mat.shape[0]
    arr = np.asarray(mat, dtype=NPBF16).reshape(r // 128, 128, *inner)
    return np.ascontiguousarray(np.moveaxis(arr, 1, 0))


_NC_CACHE = {}


def _build_nc(R):
    if R in _NC_CACHE:
        return _NC_CACHE[R]
    groups = _split_groups(R)
    NG = len(groups)

    nc = bacc.Bacc("TRN2", target_bir_lowering=False, debug=False,
                   enable_asserts=False, num_devices=NCORES)

    # all inputs pre-packed host-side to partition-major SBUF layout
    xeTs = [nc.dram_tensor(f"xeT{i}", (128, 8, gsz), BF16, kind="ExternalInput")
            for i, (g0, gsz) in enumerate(groups)]
    W1e = nc.dram_tensor("W1e", (128, 32, 8, 128), BF16, kind="ExternalInput")
    W2t = nc.dram_tensor("W2t", (128, 8, 32, 128), BF16, kind="ExternalInput")
    b1e = nc.dram_tensor("b1e", (128, 32), F32, kind="ExternalInput")
    xT = nc.dram_tensor("xT", (128, NCB, 8, 512), BF16, kind="ExternalInput")
    sW1q = nc.dram_tensor("sW1q", (128, 8, 8, 128), BF16, kind="ExternalInput")
    sW2q = nc.dram_tensor("sW2q", (128, 8, D), BF16, kind="ExternalInput")
    sb1q = nc.dram_tensor("sb1q", (128, 8), F32, kind="ExternalInput")
    yrT = nc.dram_tensor("yrT", (128, 8, R), BF16, kind="ExternalOutput")
    ys = nc.dram_tensor("ys", (T, D), BF16, kind="ExternalOutput")

    with tile.TileContext(nc) as tc:
        # Outermost pool holds everything that must live across both
        # phases; all routed-phase tensors preload during the shared
        # phase so the transition has no DMA wait.
        with tc.tile_pool(name="rpre", bufs=1) as rpre:
          sw1 = rpre.tile([128, 8, 8, 128], BF16)
          sw2 = rpre.tile([128, 8, D], BF16)
          sb1t = rpre.tile([128, 8], F32)
          xs0 = rpre.tile([128, 8, 512], BF16)
          b1t = rpre.tile([128, 32], F32)
          w1q = rpre.tile([128, 3, 8, 128], BF16)    # W1 stream ring
          w2q = rpre.tile([128, 3, 32, 128], BF16)   # W2 stream ring
          ht = rpre.tile([128, 32, R], BF16)         # routed gelu output
          xebs = []
          for i, (g0, gsz) in enumerate(groups):
              xebs.append(rpre.tile([128, 8, gsz], BF16, name=f"xeb{i}"))

          # startup-critical loads first: per-h sW1 chunks (256KB each) so
          # the first matmul waits only for h=0; tiny loads ride scalar.
          # Startup loads in staged release order. Tile hoists any
          # dependency-free DMA trigger, so every non-critical transfer
          # carries a WAW gate: a 1-element vector copy into its dst tile
          # whose src is a late slice of the previous stage. Stage 0
          # (ungated): sb1t, sw1 h0-3, xs0. Stage 1: sw1 h4-7 after xs0.
          # Stage 2: sw2 after sw1. Stage 3: all routed-phase bulk after
          # sw2. This keeps the startup-critical prefix at full HBM rate.
          # Stage 0 (ungated): the true critical prefix only.
          nc.scalar.dma_start(sb1t[:], sb1q.ap()[:])
          nc.sync.dma_start(sw1[:, 0, :, :], sW1q.ap()[:, 0, :, :])
          # xs0 streams in 2 halves (in-queue order is serial): the
          # first matmul waits only for d0-3 while d4-7 pipelines in.
          nc.gpsimd.dma_start(xs0[:, 0:4, :], xT.ap()[:, 0, 0:4, :])
          nc.gpsimd.dma_start(xs0[:, 4:8, :], xT.ap()[:, 0, 4:8, :])
          nc.sync.dma_start(sw1[:, 1:4, :, :], sW1q.ap()[:, 1:4, :, :])
          nc.vector.tensor_copy(sw1[:, 4, 0, 0:1], xs0[:, 3, 1:2])
          nc.sync.dma_start(sw1[:, 4:8, :, :], sW1q.ap()[:, 4:8, :, :])
          # Stage 2 (gated on sw1 h4-7): sw2, xs1.
          nc.vector.tensor_copy(sw2[:, 0, 0:1], sw1[:, 7, 7, 126:127])
          nc.sync.dma_start(sw2[:], sW2q.ap()[:])
          # Stage 3 (gated on sw2): all routed-phase bulk + xs2 (below).
          for i in range(NG):
              nc.vector.tensor_copy(xebs[i][:, 0, 0:1], sw2[:, 7, 1020 + i:1021 + i])
              nc.sync.dma_start(xebs[i][:], xeTs[i].ap()[:])
          for k in range(3):
              nc.vector.tensor_copy(w1q[:, k, 0, 0:1], sw2[:, 7, 1014 + k:1015 + k])
              nc.sync.dma_start(w1q[:, k, :, :], W1e.ap()[:, k, :, :])
              nc.vector.tensor_copy(w2q[:, k, 0, 0:1], sw2[:, 7, 1017 + k:1018 + k])
              nc.sync.dma_start(w2q[:, k, :, :], W2t.ap()[:, k, :, :])

          # ---------------- phase S: shared-expert slice over all tokens ----
          with tc.tile_pool(name="sxp", bufs=3) as sxp, \
             tc.tile_pool(name="shp", bufs=10) as shp, \
             tc.tile_pool(name="syp", bufs=3) as syp, \
             tc.tile_pool(name="sph", bufs=3, space="PSUM") as sph, \
             tc.tile_pool(name="spy", bufs=5, space="PSUM") as spy:
            ysr = ys.ap().rearrange("(a p) d -> p a d", p=128)
            xspre = []
            for k in (1, 2):
                xsk = sxp.tile([128, 8, 512], BF16, tag="xs", name=f"xs{k}")
                if k == 1:
                    nc.vector.tensor_copy(xsk[:, 0, 0:1], sw1[:, 7, 7, 125:126])
                else:
                    nc.vector.tensor_copy(xsk[:, 0, 0:1], sw2[:, 7, 1013:1014])
                nc.gpsimd.dma_start(xsk[:], xT.ap()[:, k, :, :])
                xspre.append(xsk)
            for cb in range(NCB):
                if cb == 0:
                    xs = xs0
                elif cb <= 2:
                    xs = xspre[cb - 1]
                else:
                    xs = sxp.tile([128, 8, 512], BF16, tag="xs")
                    nc.gpsimd.dma_start(xs[:], xT.ap()[:, cb, :, :])
                if cb == 4:
                    nc.scalar.dma_start(b1t[:], b1e.ap()[:])
                hts = []
                for h in range(8):
                    ph = sph.tile([128, 512], F32, tag="ph")
                    for d in range(8):
                        nc.tensor.matmul(ph[:], sw1[:, h, d, :],
                                         xs[:, d, :], start=(d == 0), stop=(d == 7))
                    htt = shp.tile([128, 512], BF16, tag="ht")
                    nc.scalar.activation(htt[:], ph[:], GELU, bias=sb1t[:, h:h + 1])
                    hts.append(htt)
                for cs in range(4):
                    for dh in range(2):
                        py = spy.tile([128, 512], F32, tag="py")
                        for h in range(8):
                            nc.tensor.matmul(py[:], hts[h][:, cs * 128:(cs + 1) * 128],
                                             sw2[:, h, dh * 512:(dh + 1) * 512],
                                             start=(h == 0), stop=(h == 7))
                        yt = syp.tile([128, 512], BF16, tag="yt")
                        nc.vector.tensor_copy(yt[:], py[:])
                        nc.sync.dma_start(ysr[:, cb * 4 + cs, dh * 512:(dh + 1) * 512], yt[:])

          # ---------------- phase R: routed expert -------------------------
          # GEMM1: for each of 32 hidden tiles accumulate 8 d-tiles into
          # NG group PSUMs, gelu into resident ht. W1 streams via the ring.
          with tc.tile_pool(name="rph", bufs=2, space="PSUM") as rph:
            for h in range(32):
                if 1 <= h and h + 2 < 32:
                    nc.sync.dma_start(w1q[:, (h + 2) % 3, :, :],
                                      W1e.ap()[:, h + 2, :, :])
                phs = []
                for i, (g0, gsz) in enumerate(groups):
                    phs.append(rph.tile([128, gsz], F32, tag=f"ph{i}", name=f"rph{i}"))
                for d in range(8):
                    for i, (g0, gsz) in enumerate(groups):
                        nc.tensor.matmul(phs[i][:], w1q[:, h % 3, d, :],
                                         xebs[i][:, d, :],
                                         start=(d == 0), stop=(d == 7))
                for i, (g0, gsz) in enumerate(groups):
                    nc.scalar.activation(ht[:, h, g0:g0 + gsz], phs[i][:], GELU,
                                         bias=b1t[:, h:h + 1])
          # GEMM2 transposed: stationary = W2 tile [128 h, 128 d] (streamed,
          # each used once), moving = ht[:, h, group]. Out [128 d, tokens];
          # raw y stored, gates applied host-side during scatter.
          with tc.tile_pool(name="rcp", bufs=3) as rcp, \
             tc.tile_pool(name="rpy", bufs=2, space="PSUM") as rpy:
            for dt in range(8):
                if 1 <= dt and dt + 2 < 8:
                    nc.gpsimd.dma_start(w2q[:, (dt + 2) % 3, :, :],
                                        W2t.ap()[:, dt + 2, :, :])
                pys = []
                for i, (g0, gsz) in enumerate(groups):
                    pys.append(rpy.tile([128, gsz], F32, tag=f"py{i}", name=f"rpy{i}"))
                for h in range(32):
                    for i, (g0, gsz) in enumerate(groups):
                        nc.tensor.matmul(pys[i][:], w2q[:, dt % 3, h, :],
                                         ht[:, h, g0:g0 + gsz],
                                         start=(h == 0), stop=(h == 31))
                for i, (g0, gsz) in enumerate(groups):
                    yt = rcp.tile([128, gsz], BF16, tag=f"yt{i}")
                    if dt == 7:
                        # final tile: halve the copy->store chain so the
                        # post-matmul tail is as short as possible
                        hh = gsz // 2
                        nc.vector.tensor_copy(yt[:, 0:hh], pys[i][:, 0:hh])
                        nc.sync.dma_start(yrT.ap()[:, dt, g0:g0 + hh], yt[:, 0:hh])
                        nc.vector.tensor_copy(yt[:, hh:gsz], pys[i][:, hh:gsz])
                        nc.scalar.dma_start(yrT.ap()[:, dt, g0 + hh:g0 + gsz],
                                            yt[:, hh:gsz])
                    else:
                        nc.vector.tensor_copy(yt[:], pys[i][:])
                        nc.sync.dma_start(yrT.ap()[:, dt, g0:g0 + gsz], yt[:])

    nc.compile()
    nc.finalize()
    _NC_CACHE[R] = nc
    return nc


def _route(xf, rW, rb):
    """Host router: replicates jax top_k (ties -> lower index) + softmax."""
    gates = xf @ rW + rb
    idx = np.argsort(-gates, axis=1, kind="stable")[:, :TOP_K]
    vals = np.take_along_axis(gates, idx, axis=1)
    ex = np.exp(vals - vals[:, :1])
    probs = (ex / ex.sum(axis=1, keepdims=True)).astype(np.float32)
    return idx, probs


def _run(inputs, trace=False):
    x = np.asarray(inputs["x"], dtype=np.float32)
    rW = np.asarray(inputs["rW"], dtype=np.float32)
    rb = np.asarray(inputs["rb"], dtype=np.float32)
    W1 = np.asarray(inputs["W1"], dtype=np.float32)
    b1 = np.asarray(inputs["b1"], dtype=np.float32)
    W2 = np.asarray(inputs["W2"], dtype=np.float32)
    b2 = np.asarray(inputs["b2"], dtype=np.float32)
    sW1 = np.asarray(inputs["sW1"], dtype=np.float32)
    sb1 = np.asarray(inputs["sb1"], dtype=np.float32)
    sW2 = np.asarray(inputs["sW2"], dtype=np.float32)
    sb2 = np.asarray(inputs["sb2"], dtype=np.float32)

    B, L, _ = x.shape
    xf = np.ascontiguousarray(x.reshape(-1, D))
    idx, probs = _route(xf, rW, rb)

    tok = []
    prb = []
    for e in range(E):
        sel = idx == e  # (T, K)
        rows = np.nonzero(sel.any(axis=1))[0]
        p = np.where(sel[rows, 0], probs[rows, 0], probs[rows, 1])
        tok.append(rows)
        prb.append(p.astype(np.float32))
    R = max(128, max(len(r) for r in tok))
    groups = _split_groups(R)

    nc = _build_nc(R)

    xfT16 = np.ascontiguousarray(xf.T.astype(NPBF16))       # (D, T)
    # xT packed: [p, cb, a, c] = xf[cb*512+c, a*128+p]
    xT_host = np.ascontiguousarray(
        xfT16.reshape(8, 128, NCB, 512).transpose(1, 2, 0, 3))
    b1_packed = [np.ascontiguousarray(b1[e].reshape(32, 128).T) for e in range(E)]
    in_maps = []
    for core in range(NCORES):
        s, q = core // 4, core % 4
        n_e = len(tok[core])
        xeF = np.zeros((D, R), dtype=NPBF16)
        xeF[:, :n_e] = xfT16[:, tok[core]]
        xe_blocks = {
            f"xeT{i}": np.ascontiguousarray(
                np.moveaxis(xeF[:, g0:g0 + gsz].reshape(8, 128, gsz), 1, 0))
            for i, (g0, gsz) in enumerate(groups)}
        in_maps.append({
            **xe_blocks,
            "W1e": np.ascontiguousarray(
                W1[core].astype(NPBF16).reshape(8, 128, 32, 128)
                .transpose(1, 2, 0, 3)),
            "W2t": np.ascontiguousarray(
                W2[core].astype(NPBF16).reshape(32, 128, 8, 128)
                .transpose(1, 2, 0, 3)),
            "b1e": b1_packed[core],
            "xT": xT_host,
            "sW1q": np.ascontiguousarray(
                sW1[s][:, q * HQ:(q + 1) * HQ].astype(NPBF16)
                .reshape(8, 128, 8, 128).transpose(1, 2, 0, 3)),
            "sW2q": _pack(0.5 * sW2[s][q * HQ:(q + 1) * HQ, :], (D,)),
            "sb1q": np.ascontiguousarray(sb1[s][q * HQ:(q + 1) * HQ].reshape(8, 128).T),
        })

    if trace:
        _install_ntff_hook()
    res = run_bass_kernel_spmd(nc, in_maps, list(range(NCORES)), trace=trace)

    out = np.zeros((T, D), dtype=np.float32)
    for core in range(NCORES):
        out += res.results[core]["ys"].astype(np.float32)
    out += 0.5 * (sb2[0] + sb2[1])[None, :]
    for e in range(E):
        n_e = len(tok[e])
        # yrT[p, dt, t] = y[t, dt*128+p] -> y2[t, d]
        y2 = res.results[e]["yrT"].transpose(2, 1, 0).reshape(R, D)[:n_e]
        out[tok[e]] += prb[e][:, None] * (y2.astype(np.float32) + b2[e][None, :])
    return out.reshape(B, L, D).astype(np.float32), res


def kernel(**inputs):
    out, _ = _run(inputs, trace=False)
    return out


# revision 13
# speedup vs baseline: 1.0047x; 1.0047x over previous
"""MoE feed-forward kernel for 8 Trainium2 NeuronCores.

Strategy (v2):
  - Router (tiny: x @ rW, top-2, softmax) runs on host in numpy.
  - Expert-parallel: core e owns routed expert e. Host gathers the tokens
    routed to expert e, padded only to R = max_e count_e (NOT to a
    multiple of 128): the routed phase streams exact token counts.
  - Routed GEMM2 runs in the TRANSPOSED orientation: stationary = W2
    tile [128 h, 128 d], moving = ht [128 h, tokens]. Output is
    [128 d-partitions, tokens] so cost scales with the token count, not
    with 128-aligned token blocks; the 1152-padding of v1 is gone.
    The gate multiply moves to the host scatter-add (output is raw y).
  - W1 and W2 are STREAMED through small 3-slot SBUF rings (stationary
    tiles are each used once per output pass), freeing ~48KB/partition.
  - Shared experts: sharded (expert s = core//4, hidden-quarter q =
    core%4); each core computes its quarter over all tokens; host sums
    the 8 partials (0.5 mean factor folded into sW2 upload).
  - All matmul operands bf16 (PSUM accumulates fp32).
  - Startup: sW1 is loaded in per-h 256KB chunks so the first shared
    matmul only waits for 1/8 of the weights; the warmup is 8 dummy
    matmuls (~1.7us, sized to the DMA prefix) plus one biased GELU so
    both scalar-engine activation tables load during the DMA wait.
  - Every input is pre-packed on the host into the exact SBUF tile
    layout (partition-major) so each DMA moves contiguous KBs per
    partition. Outputs are stored bf16.
"""

import sys
import types

import numpy as np
import ml_dtypes

sys.path.insert(0, "/opt/trn_rl_repo")

import concourse.bass as bass  # noqa: E402
import concourse.mybir as mybir  # noqa: E402
import concourse.tile as tile  # noqa: E402
from concourse import bacc  # noqa: E402
from concourse.bass_utils import run_bass_kernel_spmd  # noqa: E402

F32 = mybir.dt.float32
BF16 = mybir.dt.bfloat16
NPBF16 = ml_dtypes.bfloat16
GELU = mybir.ActivationFunctionType.Gelu

D = 1024      # d_model
H = 4096      # expert hidden
HQ = 1024     # shared-expert hidden slice per core (H / 4)
T = 4096      # tokens (2 * 2048)
E = 8         # routed experts
TOP_K = 2
NCORES = 8
NCB = T // 512  # token blocks in the shared phase


def _install_ntff_hook():
    """Shim for the missing antenv.axon_hooks so trace=True can profile."""
    try:
        import antenv
        if "antenv.axon_hooks" in sys.modules:
            return
        mod = types.ModuleType("antenv.axon_hooks")
        mod._hook = None
        mod.set_axon_ntff_profile_hook = lambda h: setattr(mod, "_hook", h)
        mod.get_axon_ntff_profile_hook = lambda: mod._hook
        sys.modules["antenv.axon_hooks"] = mod
        antenv.axon_hooks = mod
        sys.path.insert(0, "/root/.axon_site/trn_agent_boot")
        import trn_boot
        hook = trn_boot._ntff_profile_via_ctypes("/opt/axon/libaxon_pjrt.so")
        mod.set_axon_ntff_profile_hook(hook)
    except Exception:
        pass


def _split_groups(r):
    """Split r tokens into balanced moving-dim groups of <=512 (each >=64
    so LDWEIGHTS stays hidden under the stream)."""
    n = (r + 511) // 512
    base = r // n
    rem = r - base * n
    out = []
    start = 0
    for i in range(n):
        sz = base + (1 if i < rem else 0)
        out.append((start, sz))
        start += sz
    return out


def _pack(mat, inner):
    """(R, cols) -> (128, R//128, cols...) partition-major bf16 host packing:
    out[p, a, ...] = mat[a*128 + p, ...]."""
    r = mat.shape[0]
    arr = np.asarray(mat, dtype=NPBF16).reshape(r // 128, 128, *inner)
    return np.ascontiguousarray(np.moveaxis(arr, 1, 0))


_NC_CACHE = {}


def _build_nc(R):
    if R in _NC_CACHE:
        return _NC_CACHE[R]
    groups = _split_groups(R)
    NG = len(groups)

    nc = bacc.Bacc("TRN2", target_bir_lowering=False, debug=False,
                   enable_asserts=False, num_devices=NCORES)

    # all inputs pre-packed host-side to partition-major SBUF layout
    xeTs = [nc.dram_tensor(f"xeT{i}", (128, 8, gsz), BF16, kind="ExternalInput")
            for i, (g0, gsz) in enumerate(groups)]
    W1e = nc.dram_tensor("W1e", (128, 32, 8, 128), BF16, kind="ExternalInput")
    W2t = nc.dram_tensor("W2t", (128, 8, 32, 128), BF16, kind="ExternalInput")
    b1e = nc.dram_tensor("b1e", (128, 32), F32, kind="ExternalInput")
    xT = nc.dram_tensor("xT", (128, NCB, 8, 512), BF16, kind="ExternalInput")
    sW1q = nc.dram_tensor("sW1q", (128, 8, 8, 128), BF16, kind="ExternalInput")
    sW2q = nc.dram_tensor("sW2q", (128, 8, D), BF16, kind="ExternalInput")
    sb1q = nc.dram_tensor("sb1q", (128, 8), F32, kind="ExternalInput")
    yrT = nc.dram_tensor("yrT", (128, 8, R), BF16, kind="ExternalOutput")
    ys = nc.dram_tensor("ys", (T, D), BF16, kind="ExternalOutput")

    with tile.TileContext(nc) as tc:
        # Outermost pool holds everything that must live across both
        # phases; all routed-phase tensors preload during the shared
        # phase so the transition has no DMA wait.
        with tc.tile_pool(name="rpre", bufs=1) as rpre:
          sw1 = rpre.tile([128, 8, 8, 128], BF16)
          sw2 = rpre.tile([128, 8, D], BF16)
          sb1t = rpre.tile([128, 8], F32)
          xs0 = rpre.tile([128, 8, 512], BF16)
          b1t = rpre.tile([128, 32], F32)
          w1q = rpre.tile([128, 3, 8, 128], BF16)    # W1 stream ring
          w2q = rpre.tile([128, 3, 32, 128], BF16)   # W2 stream ring
          ht = rpre.tile([128, 32, R], BF16)         # routed gelu output
          xebs = []
          for i, (g0, gsz) in enumerate(groups):
              xebs.append(rpre.tile([128, 8, gsz], BF16, name=f"xeb{i}"))

          # startup-critical loads first: per-h sW1 chunks (256KB each) so
          # the first matmul waits only for h=0; tiny loads ride scalar.
          # Startup loads in staged release order. Tile hoists any
          # dependency-free DMA trigger, so every non-critical transfer
          # carries a WAW gate: a 1-element vector copy into its dst tile
          # whose src is a late slice of the previous stage. Stage 0
          # (ungated): sb1t, sw1 h0-3, xs0. Stage 1: sw1 h4-7 after xs0.
          # Stage 2: sw2 after sw1. Stage 3: all routed-phase bulk after
          # sw2. This keeps the startup-critical prefix at full HBM rate.
          # Stage 0 (ungated): the true critical prefix only.
          nc.scalar.dma_start(sb1t[:], sb1q.ap()[:])
          nc.sync.dma_start(sw1[:, 0, :, :], sW1q.ap()[:, 0, :, :])
          # xs0 streams in 2 halves (in-queue order is serial): the
          # first matmul waits only for d0-3 while d4-7 pipelines in.
          nc.gpsimd.dma_start(xs0[:, 0:4, :], xT.ap()[:, 0, 0:4, :])
          nc.gpsimd.dma_start(xs0[:, 4:8, :], xT.ap()[:, 0, 4:8, :])
          nc.sync.dma_start(sw1[:, 1:4, :, :], sW1q.ap()[:, 1:4, :, :])
          nc.vector.tensor_copy(sw1[:, 4, 0, 0:1], xs0[:, 3, 1:2])
          nc.sync.dma_start(sw1[:, 4:8, :, :], sW1q.ap()[:, 4:8, :, :])
          # Stage 2 (gated on sw1 h4-7): sw2, xs1.
          nc.vector.tensor_copy(sw2[:, 0, 0:1], sw1[:, 7, 7, 126:127])
          nc.sync.dma_start(sw2[:], sW2q.ap()[:])
          # Stage 3 (gated on sw2): all routed-phase bulk + xs2 (below).
          for i in range(NG):
              nc.vector.tensor_copy(xebs[i][:, 0, 0:1], sw2[:, 7, 1020 + i:1021 + i])
              nc.sync.dma_start(xebs[i][:], xeTs[i].ap()[:])
          for k in range(3):
              nc.vector.tensor_copy(w1q[:, k, 0, 0:1], sw2[:, 7, 1014 + k:1015 + k])
              nc.sync.dma_start(w1q[:, k, :, :], W1e.ap()[:, k, :, :])
              nc.vector.tensor_copy(w2q[:, k, 0, 0:1], sw2[:, 7, 1017 + k:1018 + k])
              nc.sync.dma_start(w2q[:, k, :, :], W2t.ap()[:, k, :, :])

          # ---------------- phase S: shared-expert slice over all tokens ----
          with tc.tile_pool(name="sxp", bufs=3) as sxp, \
             tc.tile_pool(name="shp", bufs=10) as shp, \
             tc.tile_pool(name="syp", bufs=3) as syp, \
             tc.tile_pool(name="sph", bufs=3, space="PSUM") as sph, \
             tc.tile_pool(name="spy", bufs=5, space="PSUM") as spy:
            ysr = ys.ap().rearrange("(a p) d -> p a d", p=128)
            xspre = []
            for k in (1, 2):
                xsk = sxp.tile([128, 8, 512], BF16, tag="xs", name=f"xs{k}")
                if k == 1:
                    nc.vector.tensor_copy(xsk[:, 0, 0:1], sw1[:, 7, 7, 125:126])
                else:
                    nc.vector.tensor_copy(xsk[:, 0, 0:1], sw2[:, 7, 1013:1014])
                nc.gpsimd.dma_start(xsk[:], xT.ap()[:, k, :, :])
                xspre.append(xsk)
            for cb in range(NCB):
                if cb == 0:
                    xs = xs0
                elif cb <= 2:
                    xs = xspre[cb - 1]
                else:
                    xs = sxp.tile([128, 8, 512], BF16, tag="xs")
                    nc.gpsimd.dma_start(xs[:], xT.ap()[:, cb, :, :])
                if cb == 4:
                    nc.scalar.dma_start(b1t[:], b1e.ap()[:])
                hts = []
                for h in range(8):
                    ph = sph.tile([128, 512], F32, tag="ph")
                    for d in range(8):
                        nc.tensor.matmul(ph[:], sw1[:, h, d, :],
                                         xs[:, d, :], start=(d == 0), stop=(d == 7))
                    htt = shp.tile([128, 512], BF16, tag="ht")
                    nc.scalar.activation(htt[:], ph[:], GELU, bias=sb1t[:, h:h + 1])
                    hts.append(htt)
                for cs in range(4):
                    for dh in range(2):
                        py = spy.tile([128, 512], F32, tag="py")
                        for h in range(8):
                            nc.tensor.matmul(py[:], hts[h][:, cs * 128:(cs + 1) * 128],
                                             sw2[:, h, dh * 512:(dh + 1) * 512],
                                             start=(h == 0), stop=(h == 7))
                        yt = syp.tile([128, 512], BF16, tag="yt")
                        nc.vector.tensor_copy(yt[:], py[:])
                        nc.sync.dma_start(ysr[:, cb * 4 + cs, dh * 512:(dh + 1) * 512], yt[:])

          # ---------------- phase R: routed expert -------------------------
          # GEMM1: for each of 32 hidden tiles accumulate 8 d-tiles into
          # NG group PSUMs, gelu into resident ht. W1 streams via the ring.
          with tc.tile_pool(name="rph", bufs=2, space="PSUM") as rph:
            for h in range(32):
                if 1 <= h and h + 2 < 32:
                    nc.sync.dma_start(w1q[:, (h + 2) % 3, :, :],
                                      W1e.ap()[:, h + 2, :, :])
                phs = []
                for i, (g0, gsz) in enumerate(groups):
                    phs.append(rph.tile([128, gsz], F32, tag=f"ph{i}", name=f"rph{i}"))
                for d in range(8):
                    for i, (g0, gsz) in enumerate(groups):
                        nc.tensor.matmul(phs[i][:], w1q[:, h % 3, d, :],
                                         xebs[i][:, d, :],
                                         start=(d == 0), stop=(d == 7))
                for i, (g0, gsz) in enumerate(groups):
                    nc.scalar.activation(ht[:, h, g0:g0 + gsz], phs[i][:], GELU,
                                         bias=b1t[:, h:h + 1])
          # GEMM2 transposed: stationary = W2 tile [128 h, 128 d] (streamed,
          # each used once), moving = ht[:, h, group]. Out [128 d, tokens];
          # raw y stored, gates applied host-side during scatter.
          with tc.tile_pool(name="rcp", bufs=3) as rcp, \
             tc.tile_pool(name="rpy", bufs=2, space="PSUM") as rpy:
            for dt in range(8):
                if 1 <= dt and dt + 2 < 8:
                    nc.gpsimd.dma_start(w2q[:, (dt + 2) % 3, :, :],
                                        W2t.ap()[:, dt + 2, :, :])
                pys = []
                for i, (g0, gsz) in enumerate(groups):
                    pys.append(rpy.tile([128, gsz], F32, tag=f"py{i}", name=f"rpy{i}"))
                for h in range(32):
                    for i, (g0, gsz) in enumerate(groups):
                        nc.tensor.matmul(pys[i][:], w2q[:, dt % 3, h, :],
                                         ht[:, h, g0:g0 + gsz],
                                         start=(h == 0), stop=(h == 31))
                for i, (g0, gsz) in enumerate(groups):
                    yt = rcp.tile([128, gsz], BF16, tag=f"yt{i}")
                    nc.vector.tensor_copy(yt[:], pys[i][:])
                    nc.sync.dma_start(yrT.ap()[:, dt, g0:g0 + gsz], yt[:])

    nc.compile()
    nc.finalize()
    _NC_CACHE[R] = nc
    return nc


def _route(xf, rW, rb):
    """Host router: replicates jax top_k (ties -> lower index) + softmax."""
    gates = xf @ rW + rb
    idx = np.argsort(-gates, axis=1, kind="stable")[:, :TOP_K]
    vals = np.take_along_axis(gates, idx, axis=1)
    ex = np.exp(vals - vals[:, :1])
    probs = (ex / ex.sum(axis=1, keepdims=True)).astype(np.float32)
    return idx, probs


def _run(inputs, trace=False):
    x = np.asarray(inputs["x"], dtype=np.float32)
    rW = np.asarray(inputs["rW"], dtype=np.float32)
    rb = np.asarray(inputs["rb"], dtype=np.float32)
    W1 = np.asarray(inputs["W1"], dtype=np.float32)
    b1 = np.asarray(inputs["b1"], dtype=np.float32)
    W2 = np.asarray(inputs["W2"], dtype=np.float32)
    b2 = np.asarray(inputs["b2"], dtype=np.float32)
    sW1 = np.asarray(inputs["sW1"], dtype=np.float32)
    sb1 = np.asarray(inputs["sb1"], dtype=np.float32)
    sW2 = np.asarray(inputs["sW2"], dtype=np.float32)
    sb2 = np.asarray(inputs["sb2"], dtype=np.float32)

    B, L, _ = x.shape
    xf = np.ascontiguousarray(x.reshape(-1, D))
    idx, probs = _route(xf, rW, rb)

    tok = []
    prb = []
    for e in range(E):
        sel = idx == e  # (T, K)
        rows = np.nonzero(sel.any(axis=1))[0]
        p = np.where(sel[rows, 0], probs[rows, 0], probs[rows, 1])
        tok.append(rows)
        prb.append(p.astype(np.float32))
    R = max(128, max(len(r) for r in tok))
    groups = _split_groups(R)

    nc = _build_nc(R)

    xfT16 = np.ascontiguousarray(xf.T.astype(NPBF16))       # (D, T)
    # xT packed: [p, cb, a, c] = xf[cb*512+c, a*128+p]
    xT_host = np.ascontiguousarray(
        xfT16.reshape(8, 128, NCB, 512).transpose(1, 2, 0, 3))
    b1_packed = [np.ascontiguousarray(b1[e].reshape(32, 128).T) for e in range(E)]
    in_maps = []
    for core in range(NCORES):
        s, q = core // 4, core % 4
        n_e = len(tok[core])
        xeF = np.zeros((D, R), dtype=NPBF16)
        xeF[:, :n_e] = xfT16[:, tok[core]]
        xe_blocks = {
            f"xeT{i}": np.ascontiguousarray(
                np.moveaxis(xeF[:, g0:g0 + gsz].reshape(8, 128, gsz), 1, 0))
            for i, (g0, gsz) in enumerate(groups)}
        in_maps.append({
            **xe_blocks,
            "W1e": np.ascontiguousarray(
                W1[core].astype(NPBF16).reshape(8, 128, 32, 128)
                .transpose(1, 2, 0, 3)),
            "W2t": np.ascontiguousarray(
                W2[core].astype(NPBF16).reshape(32, 128, 8, 128)
                .transpose(1, 2, 0, 3)),
            "b1e": b1_packed[core],
            "xT": xT_host,
            "sW1q": np.ascontiguousarray(
                sW1[s][:, q * HQ:(q + 1) * HQ].astype(NPBF16)
                .reshape(8, 128, 8, 128).transpose(1, 2, 0, 3)),
            "sW2q": _pack(0.5 * sW2[s][q * HQ:(q + 1) * HQ, :], (D,)),
            "sb1q": np.ascontiguousarray(sb1[s][q * HQ:(q + 1) * HQ].reshape(8, 128).T),
        })

    if trace:
        _install_ntff_hook()
    res = run_bass_kernel_spmd(nc, in_maps, list(range(NCORES)), trace=trace)

    out = np.zeros((T, D), dtype=np.float32)
    for core in range(NCORES):
        out += res.results[core]["ys"].astype(np.float32)
    out += 0.5 * (sb2[0] + sb2[1])[None, :]
    for e in range(E):
        n_e = len(tok[e])
        # yrT[p, dt, t] = y[t, dt*128+p] -> y2[t, d]
        y2 = res.results[e]["yrT"].transpose(2, 1, 0).reshape(R, D)[:n_e]
        out[tok[e]] += prb[e][:, None] * (y2.astype(np.float32) + b2[e][None, :])
    return out.reshape(B, L, D).astype(np.float32), res


def kernel(**inputs):
    out, _ = _run(inputs, trace=False)
    return out


# revision 15
# speedup vs baseline: 1.0055x; 1.0008x over previous
"""MoE feed-forward kernel for 8 Trainium2 NeuronCores.

Strategy (v2):
  - Router (tiny: x @ rW, top-2, softmax) runs on host in numpy.
  - Expert-parallel: core e owns routed expert e. Host gathers the tokens
    routed to expert e, padded only to R = max_e count_e (NOT to a
    multiple of 128): the routed phase streams exact token counts.
  - Routed GEMM2 runs in the TRANSPOSED orientation: stationary = W2
    tile [128 h, 128 d], moving = ht [128 h, tokens]. Output is
    [128 d-partitions, tokens] so cost scales with the token count, not
    with 128-aligned token blocks; the 1152-padding of v1 is gone.
    The gate multiply moves to the host scatter-add (output is raw y).
  - W1 and W2 are STREAMED through small 3-slot SBUF rings (stationary
    tiles are each used once per output pass), freeing ~48KB/partition.
  - Shared experts: sharded (expert s = core//4, hidden-quarter q =
    core%4); each core computes its quarter over all tokens; host sums
    the 8 partials (0.5 mean factor folded into sW2 upload).
  - All matmul operands bf16 (PSUM accumulates fp32).
  - Startup: sW1 is loaded in per-h 256KB chunks so the first shared
    matmul only waits for 1/8 of the weights; the warmup is 8 dummy
    matmuls (~1.7us, sized to the DMA prefix) plus one biased GELU so
    both scalar-engine activation tables load during the DMA wait.
  - Every input is pre-packed on the host into the exact SBUF tile
    layout (partition-major) so each DMA moves contiguous KBs per
    partition. Outputs are stored bf16.
"""

import sys
import types

import numpy as np
import ml_dtypes

sys.path.insert(0, "/opt/trn_rl_repo")

import concourse.bass as bass  # noqa: E402
import concourse.mybir as mybir  # noqa: E402
import concourse.tile as tile  # noqa: E402
from concourse import bacc  # noqa: E402
from concourse.bass_utils import run_bass_kernel_spmd  # noqa: E402

F32 = mybir.dt.float32
BF16 = mybir.dt.bfloat16
NPBF16 = ml_dtypes.bfloat16
GELU = mybir.ActivationFunctionType.Gelu

D = 1024      # d_model
H = 4096      # expert hidden
HQ = 1024     # shared-expert hidden slice per core (H / 4)
T = 4096      # tokens (2 * 2048)
E = 8         # routed experts
TOP_K = 2
NCORES = 8
NCB = T // 512  # token blocks in the shared phase


def _install_ntff_hook():
    """Shim for the missing antenv.axon_hooks so trace=True can profile."""
    try:
        import antenv
        if "antenv.axon_hooks" in sys.modules:
            return
        mod = types.ModuleType("antenv.axon_hooks")
        mod._hook = None
        mod.set_axon_ntff_profile_hook = lambda h: setattr(mod, "_hook", h)
        mod.get_axon_ntff_profile_hook = lambda: mod._hook
        sys.modules["antenv.axon_hooks"] = mod
        antenv.axon_hooks = mod
        sys.path.insert(0, "/root/.axon_site/trn_agent_boot")
        import trn_boot
        hook = trn_boot._ntff_profile_via_ctypes("/opt/axon/libaxon_pjrt.so")
        mod.set_axon_ntff_profile_hook(hook)
    except Exception:
        pass


def _split_groups(r):
    """Split r tokens into balanced moving-dim groups of <=512 (each >=64
    so LDWEIGHTS stays hidden under the stream)."""
    n = (r + 511) // 512
    base = r // n
    rem = r - base * n
    out = []
    start = 0
    for i in range(n):
        sz = base + (1 if i < rem else 0)
        out.append((start, sz))
        start += sz
    return out


def _pack(mat, inner):
    """(R, cols) -> (128, R//128, cols...) partition-major bf16 host packing:
    out[p, a, ...] = mat[a*128 + p, ...]."""
    r = mat.shape[0]
    arr = np.asarray(mat, dtype=NPBF16).reshape(r // 128, 128, *inner)
    return np.ascontiguousarray(np.moveaxis(arr, 1, 0))


_NC_CACHE = {}


def _build_nc(R):
    if R in _NC_CACHE:
        return _NC_CACHE[R]
    groups = _split_groups(R)
    NG = len(groups)

    nc = bacc.Bacc("TRN2", target_bir_lowering=False, debug=False,
                   enable_asserts=False, num_devices=NCORES)

    # all inputs pre-packed host-side to partition-major SBUF layout
    xeTs = [nc.dram_tensor(f"xeT{i}", (128, 8, gsz), BF16, kind="ExternalInput")
            for i, (g0, gsz) in enumerate(groups)]
    W1e = nc.dram_tensor("W1e", (128, 32, 8, 128), BF16, kind="ExternalInput")
    W2t = nc.dram_tensor("W2t", (128, 8, 32, 128), BF16, kind="ExternalInput")
    b1e = nc.dram_tensor("b1e", (128, 32), F32, kind="ExternalInput")
    xT = nc.dram_tensor("xT", (128, NCB, 8, 512), BF16, kind="ExternalInput")
    boot = nc.dram_tensor("boot", (128, 5120), BF16, kind="ExternalInput")
    sW1q = nc.dram_tensor("sW1q", (128, 8, 8, 128), BF16, kind="ExternalInput")
    sW2q = nc.dram_tensor("sW2q", (128, 8, D), BF16, kind="ExternalInput")
    sb1q = nc.dram_tensor("sb1q", (128, 8), F32, kind="ExternalInput")
    yrT = nc.dram_tensor("yrT", (128, 8, R), BF16, kind="ExternalOutput")
    ys = nc.dram_tensor("ys", (T, D), BF16, kind="ExternalOutput")

    with tile.TileContext(nc) as tc:
        # Outermost pool holds everything that must live across both
        # phases; all routed-phase tensors preload during the shared
        # phase so the transition has no DMA wait.
        with tc.tile_pool(name="rpre", bufs=1) as rpre:
          sw1 = rpre.tile([128, 8, 8, 128], BF16)
          sw2 = rpre.tile([128, 8, D], BF16)
          sb1t = rpre.tile([128, 8], F32)
          boot_t = rpre.tile([128, 5120], BF16)   # sw1[h0] (0:1024) + xs0 (1024:5120)
          b1t = rpre.tile([128, 32], F32)
          w1q = rpre.tile([128, 3, 8, 128], BF16)    # W1 stream ring
          w2q = rpre.tile([128, 3, 32, 128], BF16)   # W2 stream ring
          ht = rpre.tile([128, 32, R], BF16)         # routed gelu output
          xebs = []
          for i, (g0, gsz) in enumerate(groups):
              xebs.append(rpre.tile([128, 8, gsz], BF16, name=f"xeb{i}"))

          # startup-critical loads first: per-h sW1 chunks (256KB each) so
          # the first matmul waits only for h=0; tiny loads ride scalar.
          # Startup loads in staged release order. Tile hoists any
          # dependency-free DMA trigger, so every non-critical transfer
          # carries a WAW gate: a 1-element vector copy into its dst tile
          # whose src is a late slice of the previous stage. Stage 0
          # (ungated): sb1t, sw1 h0-3, xs0. Stage 1: sw1 h4-7 after xs0.
          # Stage 2: sw2 after sw1. Stage 3: all routed-phase bulk after
          # sw2. This keeps the startup-critical prefix at full HBM rate.
          # Stage 0 (ungated): the true critical prefix only.
          nc.scalar.dma_start(sb1t[:], sb1q.ap()[:])
          # ONE 10KB/partition contiguous transfer carries the whole
          # startup-critical set (sw1 h0 + xs0) at full descriptor rate.
          nc.gpsimd.dma_start(boot_t[:], boot.ap()[:])
          nc.sync.dma_start(sw1[:, 1:4, :, :], sW1q.ap()[:, 1:4, :, :])
          nc.vector.tensor_copy(sw1[:, 4, 0, 0:1], boot_t[:, 5119:5120])
          nc.sync.dma_start(sw1[:, 4:8, :, :], sW1q.ap()[:, 4:8, :, :])
          # Stage 2 (gated on sw1 h4-7): sw2, xs1.
          nc.vector.tensor_copy(sw2[:, 0, 0:1], sw1[:, 7, 7, 126:127])
          nc.sync.dma_start(sw2[:], sW2q.ap()[:])
          # Stage 3 (gated on sw2): all routed-phase bulk + xs2 (below).
          for i in range(NG):
              nc.vector.tensor_copy(xebs[i][:, 0, 0:1], sw2[:, 7, 1020 + i:1021 + i])
              nc.sync.dma_start(xebs[i][:], xeTs[i].ap()[:])
          for k in range(3):
              nc.vector.tensor_copy(w1q[:, k, 0, 0:1], sw2[:, 7, 1014 + k:1015 + k])
              nc.sync.dma_start(w1q[:, k, :, :], W1e.ap()[:, k, :, :])
              nc.vector.tensor_copy(w2q[:, k, 0, 0:1], sw2[:, 7, 1017 + k:1018 + k])
              nc.sync.dma_start(w2q[:, k, :, :], W2t.ap()[:, k, :, :])

          # ---------------- phase S: shared-expert slice over all tokens ----
          with tc.tile_pool(name="sxp", bufs=3) as sxp, \
             tc.tile_pool(name="shp", bufs=10) as shp, \
             tc.tile_pool(name="syp", bufs=3) as syp, \
             tc.tile_pool(name="sph", bufs=3, space="PSUM") as sph, \
             tc.tile_pool(name="spy", bufs=5, space="PSUM") as spy:
            ysr = ys.ap().rearrange("(a p) d -> p a d", p=128)
            xspre = []
            for k in (1, 2):
                xsk = sxp.tile([128, 8, 512], BF16, tag="xs", name=f"xs{k}")
                if k == 1:
                    nc.vector.tensor_copy(xsk[:, 0, 0:1], sw1[:, 7, 7, 125:126])
                else:
                    nc.vector.tensor_copy(xsk[:, 0, 0:1], sw2[:, 7, 1013:1014])
                nc.gpsimd.dma_start(xsk[:], xT.ap()[:, k, :, :])
                xspre.append(xsk)
            for cb in range(NCB):
                if cb == 0:
                    xs = None
                elif cb <= 2:
                    xs = xspre[cb - 1]
                else:
                    xs = sxp.tile([128, 8, 512], BF16, tag="xs")
                    nc.gpsimd.dma_start(xs[:], xT.ap()[:, cb, :, :])
                if cb == 4:
                    nc.scalar.dma_start(b1t[:], b1e.ap()[:])
                hts = []
                for h in range(8):
                    ph = sph.tile([128, 512], F32, tag="ph")
                    for d in range(8):
                        if h == 0:
                            lhs = boot_t[:, d * 128:(d + 1) * 128]
                        else:
                            lhs = sw1[:, h, d, :]
                        if cb == 0:
                            rhs = boot_t[:, 1024 + d * 512:1024 + (d + 1) * 512]
                        else:
                            rhs = xs[:, d, :]
                        nc.tensor.matmul(ph[:], lhs, rhs,
                                         start=(d == 0), stop=(d == 7))
                    htt = shp.tile([128, 512], BF16, tag="ht")
                    nc.scalar.activation(htt[:], ph[:], GELU, bias=sb1t[:, h:h + 1])
                    hts.append(htt)
                for cs in range(4):
                    for dh in range(2):
                        py = spy.tile([128, 512], F32, tag="py")
                        for h in range(8):
                            nc.tensor.matmul(py[:], hts[h][:, cs * 128:(cs + 1) * 128],
                                             sw2[:, h, dh * 512:(dh + 1) * 512],
                                             start=(h == 0), stop=(h == 7))
                        yt = syp.tile([128, 512], BF16, tag="yt")
                        nc.vector.tensor_copy(yt[:], py[:])
                        nc.sync.dma_start(ysr[:, cb * 4 + cs, dh * 512:(dh + 1) * 512], yt[:])

          # ---------------- phase R: routed expert -------------------------
          # GEMM1: for each of 32 hidden tiles accumulate 8 d-tiles into
          # NG group PSUMs, gelu into resident ht. W1 streams via the ring.
          with tc.tile_pool(name="rph", bufs=2, space="PSUM") as rph:
            for h in range(32):
                if 1 <= h and h + 2 < 32:
                    nc.sync.dma_start(w1q[:, (h + 2) % 3, :, :],
                                      W1e.ap()[:, h + 2, :, :])
                phs = []
                for i, (g0, gsz) in enumerate(groups):
                    phs.append(rph.tile([128, gsz], F32, tag=f"ph{i}", name=f"rph{i}"))
                for d in range(8):
                    for i, (g0, gsz) in enumerate(groups):
                        nc.tensor.matmul(phs[i][:], w1q[:, h % 3, d, :],
                                         xebs[i][:, d, :],
                                         start=(d == 0), stop=(d == 7))
                for i, (g0, gsz) in enumerate(groups):
                    nc.scalar.activation(ht[:, h, g0:g0 + gsz], phs[i][:], GELU,
                                         bias=b1t[:, h:h + 1])
          # GEMM2 transposed: stationary = W2 tile [128 h, 128 d] (streamed,
          # each used once), moving = ht[:, h, group]. Out [128 d, tokens];
          # raw y stored, gates applied host-side during scatter.
          with tc.tile_pool(name="rcp", bufs=3) as rcp, \
             tc.tile_pool(name="rpy", bufs=2, space="PSUM") as rpy:
            for dt in range(8):
                if 1 <= dt and dt + 2 < 8:
                    nc.gpsimd.dma_start(w2q[:, (dt + 2) % 3, :, :],
                                        W2t.ap()[:, dt + 2, :, :])
                pys = []
                for i, (g0, gsz) in enumerate(groups):
                    pys.append(rpy.tile([128, gsz], F32, tag=f"py{i}", name=f"rpy{i}"))
                for h in range(32):
                    for i, (g0, gsz) in enumerate(groups):
                        nc.tensor.matmul(pys[i][:], w2q[:, dt % 3, h, :],
                                         ht[:, h, g0:g0 + gsz],
                                         start=(h == 0), stop=(h == 31))
                for i, (g0, gsz) in enumerate(groups):
                    yt = rcp.tile([128, gsz], BF16, tag=f"yt{i}")
                    nc.vector.tensor_copy(yt[:], pys[i][:])
                    nc.sync.dma_start(yrT.ap()[:, dt, g0:g0 + gsz], yt[:])

    nc.compile()
    nc.finalize()
    _NC_CACHE[R] = nc
    return nc


def _route(xf, rW, rb):
    """Host router: replicates jax top_k (ties -> lower index) + softmax."""
    gates = xf @ rW + rb
    idx = np.argsort(-gates, axis=1, kind="stable")[:, :TOP_K]
    vals = np.take_along_axis(gates, idx, axis=1)
    ex = np.exp(vals - vals[:, :1])
    probs = (ex / ex.sum(axis=1, keepdims=True)).astype(np.float32)
    return idx, probs


def _run(inputs, trace=False):
    x = np.asarray(inputs["x"], dtype=np.float32)
    rW = np.asarray(inputs["rW"], dtype=np.float32)
    rb = np.asarray(inputs["rb"], dtype=np.float32)
    W1 = np.asarray(inputs["W1"], dtype=np.float32)
    b1 = np.asarray(inputs["b1"], dtype=np.float32)
    W2 = np.asarray(inputs["W2"], dtype=np.float32)
    b2 = np.asarray(inputs["b2"], dtype=np.float32)
    sW1 = np.asarray(inputs["sW1"], dtype=np.float32)
    sb1 = np.asarray(inputs["sb1"], dtype=np.float32)
    sW2 = np.asarray(inputs["sW2"], dtype=np.float32)
    sb2 = np.asarray(inputs["sb2"], dtype=np.float32)

    B, L, _ = x.shape
    xf = np.ascontiguousarray(x.reshape(-1, D))
    idx, probs = _route(xf, rW, rb)

    tok = []
    prb = []
    for e in range(E):
        sel = idx == e  # (T, K)
        rows = np.nonzero(sel.any(axis=1))[0]
        p = np.where(sel[rows, 0], probs[rows, 0], probs[rows, 1])
        tok.append(rows)
        prb.append(p.astype(np.float32))
    R = max(128, max(len(r) for r in tok))
    groups = _split_groups(R)

    nc = _build_nc(R)

    xfT16 = np.ascontiguousarray(xf.T.astype(NPBF16))       # (D, T)
    # xT packed: [p, cb, a, c] = xf[cb*512+c, a*128+p]
    xT_host = np.ascontiguousarray(
        xfT16.reshape(8, 128, NCB, 512).transpose(1, 2, 0, 3))
    b1_packed = [np.ascontiguousarray(b1[e].reshape(32, 128).T) for e in range(E)]
    in_maps = []
    for core in range(NCORES):
        s, q = core // 4, core % 4
        n_e = len(tok[core])
        xeF = np.zeros((D, R), dtype=NPBF16)
        xeF[:, :n_e] = xfT16[:, tok[core]]
        xe_blocks = {
            f"xeT{i}": np.ascontiguousarray(
                np.moveaxis(xeF[:, g0:g0 + gsz].reshape(8, 128, gsz), 1, 0))
            for i, (g0, gsz) in enumerate(groups)}
        sW1q_arr = np.ascontiguousarray(
            sW1[s][:, q * HQ:(q + 1) * HQ].astype(NPBF16)
            .reshape(8, 128, 8, 128).transpose(1, 2, 0, 3))
        boot_arr = np.concatenate(
            [sW1q_arr[:, 0].reshape(128, 1024),
             xT_host[:, 0].reshape(128, 4096)], axis=1)
        in_maps.append({
            **xe_blocks,
            "boot": np.ascontiguousarray(boot_arr),
            "W1e": np.ascontiguousarray(
                W1[core].astype(NPBF16).reshape(8, 128, 32, 128)
                .transpose(1, 2, 0, 3)),
            "W2t": np.ascontiguousarray(
                W2[core].astype(NPBF16).reshape(32, 128, 8, 128)
                .transpose(1, 2, 0, 3)),
            "b1e": b1_packed[core],
            "xT": xT_host,
            "sW1q": sW1q_arr,
            "sW2q": _pack(0.5 * sW2[s][q * HQ:(q + 1) * HQ, :], (D,)),
            "sb1q": np.ascontiguousarray(sb1[s][q * HQ:(q + 1) * HQ].reshape(8, 128).T),
        })

    if trace:
        _install_ntff_hook()
    res = run_bass_kernel_spmd(nc, in_maps, list(range(NCORES)), trace=trace)

    out = np.zeros((T, D), dtype=np.float32)
    for core in range(NCORES):
        out += res.results[core]["ys"].astype(np.float32)
    out += 0.5 * (sb2[0] + sb2[1])[None, :]
    for e in range(E):
        n_e = len(tok[e])
        # yrT[p, dt, t] = y[t, dt*128+p] -> y2[t, d]
        y2 = res.results[e]["yrT"].transpose(2, 1, 0).reshape(R, D)[:n_e]
        out[tok[e]] += prb[e][:, None] * (y2.astype(np.float32) + b2[e][None, :])
    return out.reshape(B, L, D).astype(np.float32), res


def kernel(**inputs):
    out, _ = _run(inputs, trace=False)
    return out


# revision 16
# speedup vs baseline: 1.0071x; 1.0016x over previous
"""MoE feed-forward kernel for 8 Trainium2 NeuronCores.

Strategy (v2):
  - Router (tiny: x @ rW, top-2, softmax) runs on host in numpy.
  - Expert-parallel: core e owns routed expert e. Host gathers the tokens
    routed to expert e, padded only to R = max_e count_e (NOT to a
    multiple of 128): the routed phase streams exact token counts.
  - Routed GEMM2 runs in the TRANSPOSED orientation: stationary = W2
    tile [128 h, 128 d], moving = ht [128 h, tokens]. Output is
    [128 d-partitions, tokens] so cost scales with the token count, not
    with 128-aligned token blocks; the 1152-padding of v1 is gone.
    The gate multiply moves to the host scatter-add (output is raw y).
  - W1 and W2 are STREAMED through small 3-slot SBUF rings (stationary
    tiles are each used once per output pass), freeing ~48KB/partition.
  - Shared experts: sharded (expert s = core//4, hidden-quarter q =
    core%4); each core computes its quarter over all tokens; host sums
    the 8 partials (0.5 mean factor folded into sW2 upload).
  - All matmul operands bf16 (PSUM accumulates fp32).
  - Startup: sW1 is loaded in per-h 256KB chunks so the first shared
    matmul only waits for 1/8 of the weights; the warmup is 8 dummy
    matmuls (~1.7us, sized to the DMA prefix) plus one biased GELU so
    both scalar-engine activation tables load during the DMA wait.
  - Every input is pre-packed on the host into the exact SBUF tile
    layout (partition-major) so each DMA moves contiguous KBs per
    partition. Outputs are stored bf16.
"""

import sys
import types

import numpy as np
import ml_dtypes

sys.path.insert(0, "/opt/trn_rl_repo")

import concourse.bass as bass  # noqa: E402
import concourse.mybir as mybir  # noqa: E402
import concourse.tile as tile  # noqa: E402
from concourse import bacc  # noqa: E402
from concourse.bass_utils import run_bass_kernel_spmd  # noqa: E402

F32 = mybir.dt.float32
BF16 = mybir.dt.bfloat16
NPBF16 = ml_dtypes.bfloat16
GELU = mybir.ActivationFunctionType.Gelu

D = 1024      # d_model
H = 4096      # expert hidden
HQ = 1024     # shared-expert hidden slice per core (H / 4)
T = 4096      # tokens (2 * 2048)
E = 8         # routed experts
TOP_K = 2
NCORES = 8
NCB = T // 512  # token blocks in the shared phase


def _install_ntff_hook():
    """Shim for the missing antenv.axon_hooks so trace=True can profile."""
    try:
        import antenv
        if "antenv.axon_hooks" in sys.modules:
            return
        mod = types.ModuleType("antenv.axon_hooks")
        mod._hook = None
        mod.set_axon_ntff_profile_hook = lambda h: setattr(mod, "_hook", h)
        mod.get_axon_ntff_profile_hook = lambda: mod._hook
        sys.modules["antenv.axon_hooks"] = mod
        antenv.axon_hooks = mod
        sys.path.insert(0, "/root/.axon_site/trn_agent_boot")
        import trn_boot
        hook = trn_boot._ntff_profile_via_ctypes("/opt/axon/libaxon_pjrt.so")
        mod.set_axon_ntff_profile_hook(hook)
    except Exception:
        pass


def _split_groups(r):
    """Split r tokens into balanced moving-dim groups of <=512 (each >=64
    so LDWEIGHTS stays hidden under the stream)."""
    n = (r + 511) // 512
    base = r // n
    rem = r - base * n
    out = []
    start = 0
    for i in range(n):
        sz = base + (1 if i < rem else 0)
        out.append((start, sz))
        start += sz
    return out


def _pack(mat, inner):
    """(R, cols) -> (128, R//128, cols...) partition-major bf16 host packing:
    out[p, a, ...] = mat[a*128 + p, ...]."""
    r = mat.shape[0]
    arr = np.asarray(mat, dtype=NPBF16).reshape(r // 128, 128, *inner)
    return np.ascontiguousarray(np.moveaxis(arr, 1, 0))


_NC_CACHE = {}


def _build_nc(R):
    if R in _NC_CACHE:
        return _NC_CACHE[R]
    groups = _split_groups(R)
    NG = len(groups)

    nc = bacc.Bacc("TRN2", target_bir_lowering=False, debug=False,
                   enable_asserts=False, num_devices=NCORES)

    # all inputs pre-packed host-side to partition-major SBUF layout
    xeTs = [nc.dram_tensor(f"xeT{i}", (128, 8, gsz), BF16, kind="ExternalInput")
            for i, (g0, gsz) in enumerate(groups)]
    W1e = nc.dram_tensor("W1e", (128, 32, 8, 128), BF16, kind="ExternalInput")
    W2t = nc.dram_tensor("W2t", (128, 8, 32, 128), BF16, kind="ExternalInput")
    b1e = nc.dram_tensor("b1e", (128, 32), F32, kind="ExternalInput")
    xT = nc.dram_tensor("xT", (128, NCB, 8, 512), BF16, kind="ExternalInput")
    sW1q = nc.dram_tensor("sW1q", (128, 8, 8, 128), BF16, kind="ExternalInput")
    sW2q = nc.dram_tensor("sW2q", (128, 8, D), BF16, kind="ExternalInput")
    sb1q = nc.dram_tensor("sb1q", (128, 8), F32, kind="ExternalInput")
    yrT = nc.dram_tensor("yrT", (128, 8, R), BF16, kind="ExternalOutput")
    ys = nc.dram_tensor("ys", (T, D), BF16, kind="ExternalOutput")

    with tile.TileContext(nc) as tc:
        # Outermost pool holds everything that must live across both
        # phases; all routed-phase tensors preload during the shared
        # phase so the transition has no DMA wait.
        with tc.tile_pool(name="rpre", bufs=1) as rpre:
          sw1 = rpre.tile([128, 8, 8, 128], BF16)
          sw2 = rpre.tile([128, 8, D], BF16)
          sb1t = rpre.tile([128, 8], F32)
          xs0 = rpre.tile([128, 8, 512], BF16)
          b1t = rpre.tile([128, 32], F32)
          w1q = rpre.tile([128, 3, 8, 128], BF16)    # W1 stream ring
          w2q = rpre.tile([128, 3, 32, 128], BF16)   # W2 stream ring
          ht = rpre.tile([128, 32, R], BF16)         # routed gelu output
          xebs = []
          for i, (g0, gsz) in enumerate(groups):
              xebs.append(rpre.tile([128, 8, gsz], BF16, name=f"xeb{i}"))

          # startup-critical loads first: per-h sW1 chunks (256KB each) so
          # the first matmul waits only for h=0; tiny loads ride scalar.
          # Startup loads in staged release order. Tile hoists any
          # dependency-free DMA trigger, so every non-critical transfer
          # carries a WAW gate: a 1-element vector copy into its dst tile
          # whose src is a late slice of the previous stage. Stage 0
          # (ungated): sb1t, sw1 h0-3, xs0. Stage 1: sw1 h4-7 after xs0.
          # Stage 2: sw2 after sw1. Stage 3: all routed-phase bulk after
          # sw2. This keeps the startup-critical prefix at full HBM rate.
          # Stage 0 (ungated): the true critical prefix only.
          nc.scalar.dma_start(sb1t[:], sb1q.ap()[:])
          nc.sync.dma_start(sw1[:, 0, :, :], sW1q.ap()[:, 0, :, :])
          nc.gpsimd.dma_start(xs0[:, 0:4, :], xT.ap()[:, 0, 0:4, :])
          nc.gpsimd.dma_start(xs0[:, 4:8, :], xT.ap()[:, 0, 4:8, :])
          nc.sync.dma_start(sw1[:, 1:4, :, :], sW1q.ap()[:, 1:4, :, :])
          nc.vector.tensor_copy(sw1[:, 4, 0, 0:1], xs0[:, 3, 1:2])
          nc.sync.dma_start(sw1[:, 4:8, :, :], sW1q.ap()[:, 4:8, :, :])
          # Stage 2 (gated on sw1 h4-7): sw2, xs1.
          nc.vector.tensor_copy(sw2[:, 0, 0:1], sw1[:, 7, 7, 126:127])
          nc.sync.dma_start(sw2[:], sW2q.ap()[:])
          # Stage 3 (gated on sw2): all routed-phase bulk + xs2 (below).
          for i in range(NG):
              nc.vector.tensor_copy(xebs[i][:, 0, 0:1], sw2[:, 7, 1020 + i:1021 + i])
              nc.sync.dma_start(xebs[i][:], xeTs[i].ap()[:])
          for k in range(3):
              nc.vector.tensor_copy(w1q[:, k, 0, 0:1], sw2[:, 7, 1014 + k:1015 + k])
              nc.sync.dma_start(w1q[:, k, :, :], W1e.ap()[:, k, :, :])
              nc.vector.tensor_copy(w2q[:, k, 0, 0:1], sw2[:, 7, 1017 + k:1018 + k])
              nc.sync.dma_start(w2q[:, k, :, :], W2t.ap()[:, k, :, :])

          # ---------------- phase S: shared-expert slice over all tokens ----
          with tc.tile_pool(name="sxp", bufs=3) as sxp, \
             tc.tile_pool(name="shp", bufs=10) as shp, \
             tc.tile_pool(name="syp", bufs=3) as syp, \
             tc.tile_pool(name="sph", bufs=3, space="PSUM") as sph, \
             tc.tile_pool(name="spy", bufs=5, space="PSUM") as spy:
            ysr = ys.ap().rearrange("(a p) d -> p a d", p=128)
            xspre = []
            for k in (1, 2):
                xsk = sxp.tile([128, 8, 512], BF16, tag="xs", name=f"xs{k}")
                if k == 1:
                    nc.vector.tensor_copy(xsk[:, 0, 0:1], sw1[:, 7, 7, 125:126])
                else:
                    nc.vector.tensor_copy(xsk[:, 0, 0:1], sw2[:, 7, 1013:1014])
                nc.gpsimd.dma_start(xsk[:], xT.ap()[:, k, :, :])
                xspre.append(xsk)
            for cb in range(NCB):
                if cb == 0:
                    xs = xs0
                elif cb <= 2:
                    xs = xspre[cb - 1]
                else:
                    xs = sxp.tile([128, 8, 512], BF16, tag="xs")
                    nc.gpsimd.dma_start(xs[:], xT.ap()[:, cb, :, :])
                if cb == 4:
                    nc.scalar.dma_start(b1t[:], b1e.ap()[:])
                hts = []
                for h in range(8):
                    ph = sph.tile([128, 512], F32, tag="ph")
                    for d in range(8):
                        nc.tensor.matmul(ph[:], sw1[:, h, d, :],
                                         xs[:, d, :], start=(d == 0), stop=(d == 7))
                    htt = shp.tile([128, 512], BF16, tag="ht")
                    nc.scalar.activation(htt[:], ph[:], GELU, bias=sb1t[:, h:h + 1])
                    hts.append(htt)
                for cs in range(4):
                    for dh in range(2):
                        py = spy.tile([128, 512], F32, tag="py")
                        for h in range(8):
                            nc.tensor.matmul(py[:], hts[h][:, cs * 128:(cs + 1) * 128],
                                             sw2[:, h, dh * 512:(dh + 1) * 512],
                                             start=(h == 0), stop=(h == 7))
                        yt = syp.tile([128, 512], BF16, tag="yt")
                        nc.vector.tensor_copy(yt[:], py[:])
                        nc.sync.dma_start(ysr[:, cb * 4 + cs, dh * 512:(dh + 1) * 512], yt[:])

          # ---------------- phase R: routed expert -------------------------
          # GEMM1: for each of 32 hidden tiles accumulate 8 d-tiles into
          # NG group PSUMs, gelu into resident ht. W1 streams via the ring.
          with tc.tile_pool(name="rph", bufs=2, space="PSUM") as rph:
            for h in range(32):
                if 1 <= h and h + 2 < 32:
                    nc.sync.dma_start(w1q[:, (h + 2) % 3, :, :],
                                      W1e.ap()[:, h + 2, :, :])
                phs = []
                for i, (g0, gsz) in enumerate(groups):
                    phs.append(rph.tile([128, gsz], F32, tag=f"ph{i}", name=f"rph{i}"))
                for d in range(8):
                    for i, (g0, gsz) in enumerate(groups):
                        nc.tensor.matmul(phs[i][:], w1q[:, h % 3, d, :],
                                         xebs[i][:, d, :],
                                         start=(d == 0), stop=(d == 7))
                for i, (g0, gsz) in enumerate(groups):
                    nc.scalar.activation(ht[:, h, g0:g0 + gsz], phs[i][:], GELU,
                                         bias=b1t[:, h:h + 1])
          # GEMM2 transposed: stationary = W2 tile [128 h, 128 d] (streamed,
          # each used once), moving = ht[:, h, group]. Out [128 d, tokens];
          # raw y stored, gates applied host-side during scatter.
          with tc.tile_pool(name="rcp", bufs=3) as rcp, \
             tc.tile_pool(name="rpy", bufs=2, space="PSUM") as rpy:
            for dt in range(8):
                if 1 <= dt and dt + 2 < 8:
                    nc.gpsimd.dma_start(w2q[:, (dt + 2) % 3, :, :],
                                        W2t.ap()[:, dt + 2, :, :])
                pys = []
                for i, (g0, gsz) in enumerate(groups):
                    pys.append(rpy.tile([128, gsz], F32, tag=f"py{i}", name=f"rpy{i}"))
                for h in range(32):
                    for i, (g0, gsz) in enumerate(groups):
                        nc.tensor.matmul(pys[i][:], w2q[:, dt % 3, h, :],
                                         ht[:, h, g0:g0 + gsz],
                                         start=(h == 0), stop=(h == 31))
                for i, (g0, gsz) in enumerate(groups):
                    yt = rcp.tile([128, gsz], BF16, tag=f"yt{i}")
                    nc.vector.tensor_copy(yt[:], pys[i][:])
                    nc.sync.dma_start(yrT.ap()[:, dt, g0:g0 + gsz], yt[:])

    nc.compile()
    nc.finalize()
    _NC_CACHE[R] = nc
    return nc


def _route(xf, rW, rb):
    """Host router: replicates jax top_k (ties -> lower index) + softmax."""
    gates = xf @ rW + rb
    idx = np.argsort(-gates, axis=1, kind="stable")[:, :TOP_K]
    vals = np.take_along_axis(gates, idx, axis=1)
    ex = np.exp(vals - vals[:, :1])
    probs = (ex / ex.sum(axis=1, keepdims=True)).astype(np.float32)
    return idx, probs


def _run(inputs, trace=False):
    x = np.asarray(inputs["x"], dtype=np.float32)
    rW = np.asarray(inputs["rW"], dtype=np.float32)
    rb = np.asarray(inputs["rb"], dtype=np.float32)
    W1 = np.asarray(inputs["W1"], dtype=np.float32)
    b1 = np.asarray(inputs["b1"], dtype=np.float32)
    W2 = np.asarray(inputs["W2"], dtype=np.float32)
    b2 = np.asarray(inputs["b2"], dtype=np.float32)
    sW1 = np.asarray(inputs["sW1"], dtype=np.float32)
    sb1 = np.asarray(inputs["sb1"], dtype=np.float32)
    sW2 = np.asarray(inputs["sW2"], dtype=np.float32)
    sb2 = np.asarray(inputs["sb2"], dtype=np.float32)

    B, L, _ = x.shape
    xf = np.ascontiguousarray(x.reshape(-1, D))
    idx, probs = _route(xf, rW, rb)

    tok = []
    prb = []
    for e in range(E):
        sel = idx == e  # (T, K)
        rows = np.nonzero(sel.any(axis=1))[0]
        p = np.where(sel[rows, 0], probs[rows, 0], probs[rows, 1])
        tok.append(rows)
        prb.append(p.astype(np.float32))
    R = max(128, max(len(r) for r in tok))
    groups = _split_groups(R)

    nc = _build_nc(R)

    xfT16 = np.ascontiguousarray(xf.T.astype(NPBF16))       # (D, T)
    # xT packed: [p, cb, a, c] = xf[cb*512+c, a*128+p]
    xT_host = np.ascontiguousarray(
        xfT16.reshape(8, 128, NCB, 512).transpose(1, 2, 0, 3))
    b1_packed = [np.ascontiguousarray(b1[e].reshape(32, 128).T) for e in range(E)]
    in_maps = []
    for core in range(NCORES):
        s, q = core // 4, core % 4
        n_e = len(tok[core])
        xeF = np.zeros((D, R), dtype=NPBF16)
        xeF[:, :n_e] = xfT16[:, tok[core]]
        xe_blocks = {
            f"xeT{i}": np.ascontiguousarray(
                np.moveaxis(xeF[:, g0:g0 + gsz].reshape(8, 128, gsz), 1, 0))
            for i, (g0, gsz) in enumerate(groups)}
        sW1q_arr = np.ascontiguousarray(
            sW1[s][:, q * HQ:(q + 1) * HQ].astype(NPBF16)
            .reshape(8, 128, 8, 128).transpose(1, 2, 0, 3))
        in_maps.append({
            **xe_blocks,
            "W1e": np.ascontiguousarray(
                W1[core].astype(NPBF16).reshape(8, 128, 32, 128)
                .transpose(1, 2, 0, 3)),
            "W2t": np.ascontiguousarray(
                W2[core].astype(NPBF16).reshape(32, 128, 8, 128)
                .transpose(1, 2, 0, 3)),
            "b1e": b1_packed[core],
            "xT": xT_host,
            "sW1q": sW1q_arr,
            "sW2q": _pack(0.5 * sW2[s][q * HQ:(q + 1) * HQ, :], (D,)),
            "sb1q": np.ascontiguousarray(sb1[s][q * HQ:(q + 1) * HQ].reshape(8, 128).T),
        })

    if trace:
        _install_ntff_hook()
    res = run_bass_kernel_spmd(nc, in_maps, list(range(NCORES)), trace=trace)

    out = np.zeros((T, D), dtype=np.float32)
    for core in range(NCORES):
        out += res.results[core]["ys"].astype(np.float32)
    out += 0.5 * (sb2[0] + sb2[1])[None, :]
    for e in range(E):
        n_e = len(tok[e])
        # yrT[p, dt, t] = y[t, dt*128+p] -> y2[t, d]
        y2 = res.results[e]["yrT"].transpose(2, 1, 0).reshape(R, D)[:n_e]
        out[tok[e]] += prb[e][:, None] * (y2.astype(np.float32) + b2[e][None, :])
    return out.reshape(B, L, D).astype(np.float32), res


def kernel(**inputs):
    out, _ = _run(inputs, trace=False)
    return out
